# revision 3
# baseline (speedup 1.0000x reference)
"""Trainium2 Bass kernel for EnhancedSelfAttention (GroupNorm + MHSA + proj + residual).

Problem: x[16, 256, 32, 32] f32; GroupNorm(1 group) -> 1x1-conv qkv (768x256)
-> 8-head self-attention over the 1024 pixels (head_dim 32) -> 1x1-conv proj
(256x256) -> + x.

Strategy: pure data parallelism over the batch, 2 samples per NeuronCore on 8
cores, no collectives. Per sample, everything stays on-chip.

v2 changes over the first working version (244.5us):
  - The softmax exp (16.8M elements/core) is the hard wall: ACT runs exp at 1
    elem/lane/cycle @1.2GHz => ~128us/core alone. The exp is therefore SPLIT
    between ACT (true exp) and DVE (one tensor_scalar op computing the
    Schraudolph approximation directly in bf16 bit space:
    i16 = round(x*(128*log2e*scale) + (127*128 - C)), bitcast int16->bf16.
    Softmax renormalization washes the sawtooth error out: full-pipeline
    numpy sim shows ~1.2e-3 rel err even with 100% approximated exp.
  - S^T matmuls are 4x-row-tiled (K=32, tile_position=(32*band, 0)), two
    concurrent per pair, operands sliced straight out of qk_sb (head h's q
    and k both live at partition band 32*(h%4)) - no kpad zero-padding and
    no kpad DMAs. Fallback flag ST_ROWTILED=False restores the K=128 kpad
    path in case row-tiled MMs fail to keep the HAM clock gate warm.
  - O evacuation copies [33,1024] per head (data + denominator row in one
    DVE op) into per-pair tiles; proj consumes the pair tiles with K=66
    matmuls whose weights have zero rows at the denominator positions.
  - Reciprocal of the softmax denominators is batched to a [32,64] tile
    (DMA reshape through DRAM scratch) instead of [4,1024] rows: the DVE
    RECIPROCAL is 8 passes/element, so free-dim size is everything.
  - The ind66 broadcast matmul result is consumed by the normalization
    multiply directly from PSUM (no bc evacuation copy).
"""

import sys

import ml_dtypes
import numpy as np

for _p in ("/opt/trn_rl_repo",):
    if _p not in sys.path:
        sys.path.insert(0, _p)

import concourse.bass as bass  # noqa: F401
import concourse.tile as tile
from concourse import bacc, mybir
from concourse.bass_utils import run_bass_kernel_spmd

BF16 = mybir.dt.bfloat16
F32 = mybir.dt.float32
F32R = mybir.dt.float32r
I16 = mybir.dt.int16
I32 = mybir.dt.int32
AF = mybir.ActivationFunctionType
OP = mybir.AluOpType

B, C, HW = 16, 256, 1024
NH, HD = 8, 32
NCORES = 8
SPC = B // NCORES  # samples per core
EPS = 1e-5
SCALE = float(HD) ** -0.5

ST_ROWTILED = True
# Schraudolph constants (bf16 bit space); conversion rounds to nearest.
A_DVE = 128.0 * 1.4426950408889634 * SCALE
B_DVE = 127.0 * 128.0 - 5.5
# (j, head-in-pair) tiles whose exp runs on DVE instead of ACT
DVE_SET = frozenset({(0, 1), (2, 1), (4, 1), (6, 1)})

_CACHE: dict = {}

_IND2 = np.zeros((128, 128), np.float32)
_IND2[0, 0:32] = 1.0
_IND2[1, 64:96] = 1.0


def _emit_gn(nc, pools, x_sb, xn_sb, gnw_sb, gnb_sb, ones_col, ones_row):
    """GroupNorm(1 group) over the full [256, 1024] sample."""
    tp, ps_misc = pools["t"], pools["ps_misc"]
    # per-partition stats over both channel chunks (2048 elems per partition)
    stat6 = tp.tile([128, 4, 6], F32, tag="stat6")
    for i in range(4):
        nc.vector.bn_stats(
            out=stat6[:, i, :], in_=x_sb[:, i // 2, 512 * (i % 2) : 512 * (i % 2) + 512]
        )
    mv = tp.tile([128, 2], F32, tag="mv")
    nc.vector.bn_aggr(out=mv, in_=stat6)
    # st2 = [mean_p, E[x^2]_p]
    st2 = tp.tile([128, 2], F32, tag="st2")
    nc.vector.tensor_copy(out=st2[:, 0:1], in_=mv[:, 0:1])
    nc.vector.scalar_tensor_tensor(
        out=st2[:, 1:2],
        in0=mv[:, 0:1],
        scalar=mv[:, 0:1],
        in1=mv[:, 1:2],
        op0=OP.mult,
        op1=OP.add,
    )
    # partition reduction: [1, 2] = ones.T @ st2
    ps_g = ps_misc.tile([1, 2], F32, tag="st")
    nc.tensor.matmul(out=ps_g, lhsT=ones_col, rhs=st2, start=True, stop=True)
    # scalars: g = (mean, E[x^2]); var = e2 - mean^2; rstd = rsqrt(var + eps)
    sc = tp.tile([1, 8], F32, tag="sc")
    nc.vector.tensor_scalar_mul(out=sc[:, 0:2], in0=ps_g, scalar1=1.0 / 128.0)
    nc.vector.tensor_mul(out=sc[:, 2:3], in0=sc[:, 0:1], in1=sc[:, 0:1])
    nc.vector.tensor_sub(out=sc[:, 3:4], in0=sc[:, 1:2], in1=sc[:, 2:3])
    nc.vector.tensor_scalar_add(out=sc[:, 4:5], in0=sc[:, 3:4], scalar1=EPS)
    vep = sc[:, 4:5]
    # Newton rsqrt seeded by the classic bit trick (robust to any scale)
    yt = tp.tile([1, 8], F32, tag="yt")
    nc.vector.tensor_scalar(
        out=yt[:, 0:1].bitcast(I32),
        in0=vep.bitcast(I32),
        scalar1=1,
        scalar2=None,
        op0=OP.logical_shift_right,
    )
    nc.vector.tensor_scalar(
        out=yt[:, 1:2].bitcast(I32),
        in0=yt[:, 0:1].bitcast(I32),
        scalar1=-1,
        scalar2=0x5F3759DF,
        op0=OP.mult,
        op1=OP.add,
    )
    y = yt[:, 1:2]
    for it in range(3):
        t0 = yt[:, 2 + it : 3 + it] if it < 2 else yt[:, 2 + (it % 2) : 3 + (it % 2)]
        nc.vector.tensor_mul(out=t0, in0=vep, in1=y)
        nc.vector.tensor_mul(out=t0, in0=t0, in1=y)
        nc.vector.tensor_scalar(
            out=t0, in0=t0, scalar1=-0.5, scalar2=1.5, op0=OP.mult, op1=OP.add
        )
        ynew = yt[:, 4 + (it % 2) : 5 + (it % 2)]
        nc.vector.tensor_mul(out=ynew, in0=y, in1=t0)
        y = ynew
    # fin = [-mean, rstd]
    fin = tp.tile([1, 2], F32, tag="fin")
    nc.vector.tensor_scalar_mul(out=fin[:, 0:1], in0=sc[:, 0:1], scalar1=-1.0)
    nc.vector.tensor_copy(out=fin[:, 1:2], in_=y)
    # broadcast to all 128 partitions
    ps_b = ps_misc.tile([128, 2], F32, tag="st")
    nc.tensor.matmul(out=ps_b, lhsT=ones_row, rhs=fin, start=True, stop=True)
    bc = tp.tile([128, 2], F32, tag="bc")
    nc.vector.tensor_copy(out=bc, in_=ps_b)
    # affine: xn = x * (rstd*gn_w) + (gn_b - mean*rstd*gn_w)
    a_sb = tp.tile([128, 2], F32, tag="asb")
    nc.vector.tensor_scalar_mul(out=a_sb, in0=gnw_sb, scalar1=bc[:, 1:2])
    b_sb = tp.tile([128, 2], F32, tag="bsb")
    nc.vector.scalar_tensor_tensor(
        out=b_sb, in0=a_sb, scalar=bc[:, 0:1], in1=gnb_sb, op0=OP.mult, op1=OP.add
    )
    for k in range(2):
        nc.vector.tensor_scalar(
            out=xn_sb[:, k, :],
            in0=x_sb[:, k, :],
            scalar1=a_sb[:, k : k + 1],
            scalar2=b_sb[:, k : k + 1],
            op0=OP.mult,
            op1=OP.add,
        )


def _build():
    nc = bacc.Bacc("TRN2", target_bir_lowering=False, debug=False)
    x_d = nc.dram_tensor("x", [SPC, C, HW], F32, kind="ExternalInput").ap()
    qkvwT_d = nc.dram_tensor("qkv_wT", [C, 3 * C], BF16, kind="ExternalInput").ap()
    qbqk_d = nc.dram_tensor("qkv_b_qk", [4, 128], F32, kind="ExternalInput").ap()
    qbv_d = nc.dram_tensor("qkv_b_v", [1, C], F32, kind="ExternalInput").ap()
    pwT128_d = nc.dram_tensor("pwT128", [128, 4 * C], BF16, kind="ExternalInput").ap()
    pb_d = nc.dram_tensor("proj_b", [2, 128], F32, kind="ExternalInput").ap()
    gnw_d = nc.dram_tensor("gn_w", [2, 128], F32, kind="ExternalInput").ap()
    gnb_d = nc.dram_tensor("gn_b", [2, 128], F32, kind="ExternalInput").ap()
    ind66_d = nc.dram_tensor("ind66", [128, 128], BF16, kind="ExternalInput").ap()
    out_d = nc.dram_tensor("out", [SPC, C, HW], F32, kind="ExternalOutput").ap()

    with tile.TileContext(nc) as tc:
        _emit(
            nc, tc, x_d, qkvwT_d, qbqk_d, qbv_d, pwT128_d, pb_d, gnw_d, gnb_d,
            ind66_d, out_d,
        )
    nc.compile()
    return nc


def _emit(
    nc, tc, x_d, qkvwT_d, qbqk_d, qbv_d, pwT128_d, pb_d, gnw_d, gnb_d, ind66_d, out_d
):
    from contextlib import ExitStack

    with ExitStack() as ctx:
        singles = ctx.enter_context(tc.tile_pool(name="singles", bufs=1))
        samp = ctx.enter_context(tc.tile_pool(name="samp", bufs=2))
        o32p = ctx.enter_context(tc.tile_pool(name="o32p", bufs=8))
        tp = ctx.enter_context(tc.tile_pool(name="small", bufs=3))
        e_pool = ctx.enter_context(tc.tile_pool(name="epool", bufs=6))
        ps_st = ctx.enter_context(tc.tile_pool(name="ps_st", bufs=3, space="PSUM"))
        ps_o = ctx.enter_context(tc.tile_pool(name="ps_o", bufs=1, space="PSUM"))
        dr = ctx.enter_context(tc.tile_pool(name="dr", bufs=3, space="DRAM"))
        ps_misc = ps_st  # transient matmul psums share the S^T slots (tag "st")
        pools = {"t": tp, "ps_misc": ps_misc}

        x_tiles = []
        for s in range(SPC):
            x_sb = samp.tile([128, 2, HW], F32, name="x_sb", tag="x")
            nc.sync.dma_start(
                out=x_sb, in_=x_d[s].rearrange("(k p) n -> p k n", p=128)
            )
            x_tiles.append(x_sb)

        # ---- kernel-lifetime constants ----
        qkvwT = singles.tile([128, 2, 3 * C], BF16)
        nc.sync.dma_start(
            out=qkvwT, in_=qkvwT_d.rearrange("(k p) o -> p k o", p=128)
        )
        pwT128 = singles.tile([128, 4, C], BF16)
        nc.sync.dma_start(out=pwT128, in_=pwT128_d.rearrange("p (b o) -> p b o", b=4))
        qb_sb = singles.tile([128, 4], F32)
        nc.sync.dma_start(out=qb_sb, in_=qbqk_d.rearrange("t p -> p t"))
        pb_sb = singles.tile([128, 2], F32)
        nc.sync.dma_start(out=pb_sb, in_=pb_d.rearrange("t p -> p t"))
        gnw_sb = singles.tile([128, 2], F32)
        nc.sync.dma_start(out=gnw_sb, in_=gnw_d.rearrange("t p -> p t"))
        gnb_sb = singles.tile([128, 2], F32)
        nc.sync.dma_start(out=gnb_sb, in_=gnb_d.rearrange("t p -> p t"))
        qbv_sb = singles.tile([1, C], F32)
        nc.sync.dma_start(out=qbv_sb, in_=qbv_d)
        ind66_sb = singles.tile([128, 128], BF16)
        nc.sync.dma_start(out=ind66_sb, in_=ind66_d)
        zeros_col = singles.tile([128, 1], F32)
        nc.vector.memset(zeros_col, 0.0)
        kpad = []
        if not ST_ROWTILED:
            for i in range(4):
                kp = singles.tile([128, HW], BF16, name=f"kpad{i}")
                nc.vector.tensor_copy(out=kp, in_=zeros_col.to_broadcast([128, HW]))
                kpad.append(kp)
        ones_col = singles.tile([128, 1], F32)
        nc.vector.memset(ones_col, 1.0)
        ones_row = singles.tile([1, 128], F32)
        nc.vector.memset(ones_row, 1.0)
        # o32 pair tiles: pre-zeroed so the proj K=128 contraction and the
        # normalization multiply see 0 (not NaN) in the unused partition rows
        o32_tiles = []
        for i in range(2 * SPC * 4):
            ot = o32p.tile([128, HW], BF16, name=f"o32_{i}", tag="o32")
            if i < SPC * 4:
                nc.vector.tensor_copy(out=ot, in_=zeros_col.to_broadcast([128, HW]))
                o32_tiles.append(ot)
        # rsi slots: [128, 1024] bf16, rows 0-1 overwritten per pair, rest zero
        rsi_slots = []
        for i in range(2):
            rs = singles.tile([128, HW], BF16, name=f"rsi{i}")
            nc.vector.tensor_copy(out=rs, in_=zeros_col.to_broadcast([128, HW]))
            rsi_slots.append(rs)
        # dummy exp: pulls the ~2.7us ACT table load off the critical path
        dummy_e = tp.tile([1, 8], F32, name="dummy_e", tag="de")
        nc.scalar.activation(out=dummy_e, in_=ones_row[:, 0:8], func=AF.Exp, scale=0.01)
        # dummy bf16 matmul burst: pre-warms the HAM clock gate during GN
        db = singles.tile([128, 512], BF16)
        nc.vector.memset(db, 0.5)
        for _i in range(16):
            pd = ps_misc.tile([64, 512], F32, name="pd", tag="st")
            nc.tensor.matmul(
                out=pd,
                lhsT=db[:, 0:64],
                rhs=db,
                start=True,
                stop=True,
                skip_group_check=True,
            )
        # broadcast of the v-part qkv bias along partitions: [128, 256]
        vb_ps = ps_misc.tile([128, C], F32, tag="st")
        nc.tensor.matmul(out=vb_ps, lhsT=ones_row, rhs=qbv_sb, start=True, stop=True)
        vb_bc = singles.tile([128, C], F32)
        nc.vector.tensor_copy(out=vb_bc, in_=vb_ps)

        xn_tiles = {}

        def emit_gn(s):
            xn_sb = samp.tile([128, 2, HW], BF16, name="xn_sb", tag="xn")
            _emit_gn(
                nc, pools, x_tiles[s], xn_sb, gnw_sb, gnb_sb, ones_col, ones_row
            )
            xn_tiles[s] = xn_sb

        def emit_qkv(s):
            xn_sb = xn_tiles[s]
            qk_sb = samp.tile([128, 4, HW], BF16, name="qk_sb", tag="qk")
            for mt in range(4):
                for hf in range(2):
                    ps = ps_misc.tile([128, 512], F32, name="ps_q", tag="st")
                    for kc in range(2):
                        nc.tensor.matmul(
                            out=ps,
                            lhsT=qkvwT[:, kc, 128 * mt : 128 * mt + 128],
                            rhs=xn_sb[:, kc, 512 * hf : 512 * hf + 512],
                            start=(kc == 0),
                            stop=(kc == 1),
                            skip_group_check=True,
                        )
                    nc.vector.tensor_scalar_add(
                        out=qk_sb[:, mt, 512 * hf : 512 * hf + 512],
                        in0=ps,
                        scalar1=qb_sb[:, mt : mt + 1],
                    )
            vn_sb = samp.tile([128, 8, NH, HD + 1], BF16, name="vn_sb", tag="vn")
            nc.vector.tensor_copy(
                out=vn_sb[:, :, :, HD : HD + 1],
                in_=ones_col.to_broadcast([128, 8, NH, 1]),
            )
            for j in range(8):
                ps = ps_misc.tile([128, C], F32, name="ps_v", tag="st")
                for kc in range(2):
                    nc.tensor.matmul(
                        out=ps,
                        lhsT=xn_sb[:, kc, 128 * j : 128 * j + 128],
                        rhs=qkvwT[:, kc, 2 * C : 3 * C],
                        start=(kc == 0),
                        stop=(kc == 1),
                        skip_group_check=True,
                    )
                nc.vector.tensor_add(
                    out=vn_sb[:, j, :, 0:HD],
                    in0=ps.rearrange("p (h d) -> p h d", h=NH),
                    in1=vb_bc.rearrange("p (h d) -> p h d", h=NH),
                )
            return qk_sb, vn_sb

        state = {}

        def emit_pair(s, pr):
            qk_sb, vn_sb = state[s]["qkv"]
            heads = (2 * pr, 2 * pr + 1)
            mq = pr // 2
            mk = 2 + mq
            if not ST_ROWTILED:
                for h in heads:
                    qbase = 32 * (h % 4)
                    nc.sync.dma_start(
                        out=kpad[h % 4][qbase : qbase + 32, :],
                        in_=qk_sb[qbase : qbase + 32, mk, :],
                    )
            o_ps = ps_o.tile([128, HW], F32, name="o_ps", tag="o")
            state[s]["ops"][pr] = o_ps
            for j in range(8):
                st_tiles = {}
                for h in heads:
                    band = h % 4
                    st = ps_st.tile([128, HW], F32, name="st", tag="st")
                    st_tiles[h] = st
                    for hf in range(2):
                        if ST_ROWTILED:
                            nc.tensor.matmul(
                                out=st[:, 512 * hf : 512 * hf + 512],
                                lhsT=qk_sb[
                                    32 * band : 32 * band + 32,
                                    mk,
                                    128 * j : 128 * j + 128,
                                ],
                                rhs=qk_sb[
                                    32 * band : 32 * band + 32,
                                    mq,
                                    512 * hf : 512 * hf + 512,
                                ],
                                start=True,
                                stop=True,
                                tile_position=(32 * band, 0),
                                skip_group_check=True,
                            )
                        else:
                            nc.tensor.matmul(
                                out=st[:, 512 * hf : 512 * hf + 512],
                                lhsT=kpad[band][:, 128 * j : 128 * j + 128],
                                rhs=qk_sb[:, mq, 512 * hf : 512 * hf + 512],
                                start=True,
                                stop=True,
                                skip_group_check=True,
                            )
                for t, h in enumerate(heads):
                    e = e_pool.tile([128, HW], BF16, name="e", tag="e")
                    if (j, t) in DVE_SET:
                        nc.vector.tensor_scalar(
                            out=e.bitcast(I16),
                            in0=st_tiles[h],
                            scalar1=A_DVE,
                            scalar2=B_DVE,
                            op0=OP.mult,
                            op1=OP.add,
                        )
                    else:
                        nc.scalar.activation(
                            out=e, in_=st_tiles[h], func=AF.Exp, scale=SCALE
                        )
                    cg = 64 * t  # column group: head A rows 0-32, head B 64-96
                    for hf in range(2):
                        nc.tensor.matmul(
                            out=o_ps[cg : cg + 33, 512 * hf : 512 * hf + 512],
                            lhsT=vn_sb[:, j, h, :],
                            rhs=e[:, 512 * hf : 512 * hf + 512],
                            start=(j == 0),
                            stop=(j == 7),
                            tile_position=(0, cg),
                            skip_group_check=True,
                        )

        def emit_pair_tail(s, pr):
            """Evacuate o_ps (data + den rows) and start the reciprocal chain."""
            o_ps = state[s]["ops"][pr]
            ot = o32_tiles[4 * s + pr]
            state[s]["o32"][pr] = ot
            # rows 0-31 head A data, 32 den A, 64-95 head B data, 96 den B
            nc.vector.tensor_copy(out=ot[0:33, :], in_=o_ps[0:33, :])
            nc.vector.tensor_copy(out=ot[64:97, :], in_=o_ps[64:97, :])
            dden = dr.tile([2, HW], BF16, name="dden", tag="dden")
            nc.sync.dma_start(out=dden[0:1, :], in_=ot[32:33, :])
            nc.sync.dma_start(out=dden[1:2, :], in_=ot[96:97, :])
            dn32 = tp.tile([32, 64], BF16, name="dn32", tag="dn32")
            nc.sync.dma_start(
                out=dn32, in_=dden.rearrange("h (p n) -> (h p) n", p=16)
            )
            rsi32 = tp.tile([32, 64], BF16, name="rsi32", tag="rsi32")
            with nc.allow_low_precision(reason="softmax denom recip in bf16"):
                nc.vector.reciprocal(out=rsi32, in_=dn32)
            drsi = dr.tile([2, HW], BF16, name="drsi", tag="drsi")
            nc.sync.dma_start(
                out=drsi.rearrange("h (p n) -> (h p) n", p=16), in_=rsi32
            )
            rslot = rsi_slots[(4 * s + pr) % 2]
            nc.sync.dma_start(out=rslot[0:2, :], in_=drsi)
            state[s]["rsi"][pr] = rslot

        def emit_norm(s, pr):
            ot = state[s]["o32"][pr]
            rslot = state[s]["rsi"][pr]
            for hf in range(2):
                bc = ps_misc.tile([128, 512], F32, name="bc", tag="st")
                nc.tensor.matmul(
                    out=bc,
                    lhsT=ind66_sb,
                    rhs=rslot[:, 512 * hf : 512 * hf + 512],
                    start=True,
                    stop=True,
                    skip_group_check=True,
                )
                nc.vector.tensor_mul(
                    out=ot[:, 512 * hf : 512 * hf + 512],
                    in0=ot[:, 512 * hf : 512 * hf + 512],
                    in1=bc,
                )

        def emit_proj(s):
            x_sb = x_tiles[s]
            out_sb = samp.tile([128, 2, HW], F32, name="out_sb", tag="outsb")
            for mt in range(2):
                for hf in range(2):
                    ps = ps_misc.tile([128, 512], F32, name="ps_p", tag="st")
                    for b in range(4):
                        nc.tensor.matmul(
                            out=ps,
                            lhsT=pwT128[:, b, 128 * mt : 128 * mt + 128],
                            rhs=state[s]["o32"][b][:, 512 * hf : 512 * hf + 512],
                            start=(b == 0),
                            stop=(b == 3),
                            skip_group_check=True,
                        )
                    nc.vector.scalar_tensor_tensor(
                        out=out_sb[:, mt, 512 * hf : 512 * hf + 512],
                        in0=ps,
                        scalar=pb_sb[:, mt : mt + 1],
                        in1=x_sb[:, mt, 512 * hf : 512 * hf + 512],
                        op0=OP.add,
                        op1=OP.add,
                    )
                    nc.sync.dma_start(
                        out=out_d[s].rearrange("(k p) n -> p k n", p=128)[
                            :, mt, 512 * hf : 512 * hf + 512
                        ],
                        in_=out_sb[:, mt, 512 * hf : 512 * hf + 512],
                    )

        for s in range(SPC):
            state[s] = {"ops": {}, "o32": {}, "rsi": {}}

        # interleaved schedule: sample 1's GN/qkv fills engine gaps during
        # sample 0's attention; each pair's normalize chain trails two pairs
        emit_gn(0)
        emit_qkv_out = emit_qkv(0)
        state[0]["qkv"] = emit_qkv_out
        emit_pair(0, 0)
        emit_pair_tail(0, 0)
        emit_pair(0, 1)
        emit_pair_tail(0, 1)
        emit_norm(0, 0)
        emit_gn(1)
        emit_pair(0, 2)
        emit_pair_tail(0, 2)
        emit_norm(0, 1)
        state[1]["qkv"] = emit_qkv(1)
        emit_pair(0, 3)
        emit_pair_tail(0, 3)
        emit_norm(0, 2)
        emit_norm(0, 3)
        emit_proj(0)
        emit_pair(1, 0)
        emit_pair_tail(1, 0)
        emit_pair(1, 1)
        emit_pair_tail(1, 1)
        emit_norm(1, 0)
        emit_pair(1, 2)
        emit_pair_tail(1, 2)
        emit_norm(1, 1)
        emit_pair(1, 3)
        emit_pair_tail(1, 3)
        emit_norm(1, 2)
        emit_norm(1, 3)
        emit_proj(1)


def _get_nc():
    if "nc" not in _CACHE:
        _CACHE["nc"] = _build()
    return _CACHE["nc"]


def _prep_shared(gn_w, gn_b, qkv_w, qkv_b, proj_w, proj_b):
    qkv_wT = np.ascontiguousarray(
        np.asarray(qkv_w, np.float32).T.astype(ml_dtypes.bfloat16)
    )
    proj_w = np.asarray(proj_w, np.float32)
    # pwT128[r, b, o]: rows 0-31 -> proj channels 64b+r, rows 64-95 ->
    # channels 64b+32+(r-64); den/junk rows (32-63, 96-127) are zero
    pwT128 = np.zeros((128, 4, C), np.float32)
    for b_ in range(4):
        pwT128[0:32, b_, :] = proj_w[:, 64 * b_ : 64 * b_ + 32].T
        pwT128[64:96, b_, :] = proj_w[:, 64 * b_ + 32 : 64 * b_ + 64].T
    qkv_b = np.asarray(qkv_b, np.float32)
    return {
        "qkv_wT": qkv_wT,
        "qkv_b_qk": np.ascontiguousarray(qkv_b[: 2 * C].reshape(4, 128)),
        "qkv_b_v": np.ascontiguousarray(qkv_b[2 * C :].reshape(1, C)),
        "pwT128": np.ascontiguousarray(
            pwT128.reshape(128, 4 * C).astype(ml_dtypes.bfloat16)
        ),
        "proj_b": np.ascontiguousarray(np.asarray(proj_b, np.float32).reshape(2, 128)),
        "gn_w": np.ascontiguousarray(np.asarray(gn_w, np.float32).reshape(2, 128)),
        "gn_b": np.ascontiguousarray(np.asarray(gn_b, np.float32).reshape(2, 128)),
        "ind66": _IND2.astype(ml_dtypes.bfloat16),
    }


def kernel(x, gn_w, gn_b, qkv_w, qkv_b, proj_w, proj_b, **_ignored):
    nc = _get_nc()
    x = np.asarray(x, dtype=np.float32).reshape(B, C, HW)
    shared = _prep_shared(gn_w, gn_b, qkv_w, qkv_b, proj_w, proj_b)
    in_maps = [
        {"x": np.ascontiguousarray(x[i * SPC : (i + 1) * SPC]), **shared}
        for i in range(NCORES)
    ]
    br = run_bass_kernel_spmd(nc, in_maps, core_ids=list(range(NCORES)))
    out = np.concatenate([r["out"] for r in br.results], axis=0)
    return out.reshape(B, C, 32, 32)


# revision 4
# speedup vs baseline: 1.0930x; 1.0930x over previous
"""Trainium2 Bass kernel for EnhancedSelfAttention (GroupNorm + MHSA + proj + residual).

Problem: x[16, 256, 32, 32] f32; GroupNorm(1 group) -> 1x1-conv qkv (768x256)
-> 8-head self-attention over the 1024 pixels (head_dim 32) -> 1x1-conv proj
(256x256) -> + x.

Strategy: pure data parallelism over the batch, 2 samples per NeuronCore on 8
cores, no collectives. Per sample, everything stays on-chip.

v2 changes over the first working version (244.5us):
  - The softmax exp (16.8M elements/core) is the hard wall: ACT runs exp at 1
    elem/lane/cycle @1.2GHz => ~128us/core alone. The exp is therefore SPLIT
    between ACT (true exp) and DVE (one tensor_scalar op computing the
    Schraudolph approximation directly in bf16 bit space:
    i16 = round(x*(128*log2e*scale) + (127*128 - C)), bitcast int16->bf16.
    Softmax renormalization washes the sawtooth error out: full-pipeline
    numpy sim shows ~1.2e-3 rel err even with 100% approximated exp.
  - S^T matmuls are 4x-row-tiled (K=32, tile_position=(32*band, 0)), two
    concurrent per pair, operands sliced straight out of qk_sb (head h's q
    and k both live at partition band 32*(h%4)) - no kpad zero-padding and
    no kpad DMAs. Fallback flag ST_ROWTILED=False restores the K=128 kpad
    path in case row-tiled MMs fail to keep the HAM clock gate warm.
  - O evacuation copies [33,1024] per head (data + denominator row in one
    DVE op) into per-pair tiles; proj consumes the pair tiles with K=66
    matmuls whose weights have zero rows at the denominator positions.
  - Reciprocal of the softmax denominators is batched to a [32,64] tile
    (DMA reshape through DRAM scratch) instead of [4,1024] rows: the DVE
    RECIPROCAL is 8 passes/element, so free-dim size is everything.
  - The ind66 broadcast matmul result is consumed by the normalization
    multiply directly from PSUM (no bc evacuation copy).
"""

import sys

import ml_dtypes
import numpy as np

for _p in ("/opt/trn_rl_repo",):
    if _p not in sys.path:
        sys.path.insert(0, _p)

import concourse.bass as bass  # noqa: F401
import concourse.tile as tile
from concourse import bacc, mybir
from concourse.bass_utils import run_bass_kernel_spmd

BF16 = mybir.dt.bfloat16
F32 = mybir.dt.float32
F32R = mybir.dt.float32r
I16 = mybir.dt.int16
I32 = mybir.dt.int32
AF = mybir.ActivationFunctionType
OP = mybir.AluOpType

B, C, HW = 16, 256, 1024
NH, HD = 8, 32
NCORES = 8
SPC = B // NCORES  # samples per core
EPS = 1e-5
SCALE = float(HD) ** -0.5

ST_ROWTILED = False
# Schraudolph constants (bf16 bit space); conversion rounds to nearest.
A_DVE = 128.0 * 1.4426950408889634 * SCALE
B_DVE = 127.0 * 128.0 - 5.5
# (j, head-in-pair) tiles whose exp runs on DVE instead of ACT
DVE_SET = frozenset({(0, 1), (2, 1), (4, 1), (6, 1)})

_CACHE: dict = {}

_IND2 = np.zeros((128, 128), np.float32)
_IND2[0, 0:32] = 1.0
_IND2[1, 64:96] = 1.0


def _emit_gn(nc, pools, x_sb, xn_sb, gnw_sb, gnb_sb, ones_col, ones_row):
    """GroupNorm(1 group) over the full [256, 1024] sample."""
    tp, ps_misc = pools["t"], pools["ps_misc"]
    # per-partition stats over both channel chunks (2048 elems per partition)
    stat6 = tp.tile([128, 4, 6], F32, tag="stat6")
    for i in range(4):
        nc.vector.bn_stats(
            out=stat6[:, i, :], in_=x_sb[:, i // 2, 512 * (i % 2) : 512 * (i % 2) + 512]
        )
    mv = tp.tile([128, 2], F32, tag="mv")
    nc.vector.bn_aggr(out=mv, in_=stat6)
    # st2 = [mean_p, E[x^2]_p]
    st2 = tp.tile([128, 2], F32, tag="st2")
    nc.vector.tensor_copy(out=st2[:, 0:1], in_=mv[:, 0:1])
    nc.vector.scalar_tensor_tensor(
        out=st2[:, 1:2],
        in0=mv[:, 0:1],
        scalar=mv[:, 0:1],
        in1=mv[:, 1:2],
        op0=OP.mult,
        op1=OP.add,
    )
    # partition reduction: [1, 2] = ones.T @ st2
    ps_g = ps_misc.tile([1, 2], F32, tag="st")
    nc.tensor.matmul(out=ps_g, lhsT=ones_col, rhs=st2, start=True, stop=True)
    # scalars: g = (mean, E[x^2]); var = e2 - mean^2; rstd = rsqrt(var + eps)
    sc = tp.tile([1, 8], F32, tag="sc")
    nc.vector.tensor_scalar_mul(out=sc[:, 0:2], in0=ps_g, scalar1=1.0 / 128.0)
    nc.vector.tensor_mul(out=sc[:, 2:3], in0=sc[:, 0:1], in1=sc[:, 0:1])
    nc.vector.tensor_sub(out=sc[:, 3:4], in0=sc[:, 1:2], in1=sc[:, 2:3])
    nc.vector.tensor_scalar_add(out=sc[:, 4:5], in0=sc[:, 3:4], scalar1=EPS)
    vep = sc[:, 4:5]
    # Newton rsqrt seeded by the classic bit trick (robust to any scale)
    yt = tp.tile([1, 8], F32, tag="yt")
    nc.vector.tensor_scalar(
        out=yt[:, 0:1].bitcast(I32),
        in0=vep.bitcast(I32),
        scalar1=1,
        scalar2=None,
        op0=OP.logical_shift_right,
    )
    nc.vector.tensor_scalar(
        out=yt[:, 1:2].bitcast(I32),
        in0=yt[:, 0:1].bitcast(I32),
        scalar1=-1,
        scalar2=0x5F3759DF,
        op0=OP.mult,
        op1=OP.add,
    )
    y = yt[:, 1:2]
    for it in range(3):
        t0 = yt[:, 2 + it : 3 + it] if it < 2 else yt[:, 2 + (it % 2) : 3 + (it % 2)]
        nc.vector.tensor_mul(out=t0, in0=vep, in1=y)
        nc.vector.tensor_mul(out=t0, in0=t0, in1=y)
        nc.vector.tensor_scalar(
            out=t0, in0=t0, scalar1=-0.5, scalar2=1.5, op0=OP.mult, op1=OP.add
        )
        ynew = yt[:, 4 + (it % 2) : 5 + (it % 2)]
        nc.vector.tensor_mul(out=ynew, in0=y, in1=t0)
        y = ynew
    # fin = [-mean, rstd]
    fin = tp.tile([1, 2], F32, tag="fin")
    nc.vector.tensor_scalar_mul(out=fin[:, 0:1], in0=sc[:, 0:1], scalar1=-1.0)
    nc.vector.tensor_copy(out=fin[:, 1:2], in_=y)
    # broadcast to all 128 partitions
    ps_b = ps_misc.tile([128, 2], F32, tag="st")
    nc.tensor.matmul(out=ps_b, lhsT=ones_row, rhs=fin, start=True, stop=True)
    bc = tp.tile([128, 2], F32, tag="bc")
    nc.vector.tensor_copy(out=bc, in_=ps_b)
    # affine: xn = x * (rstd*gn_w) + (gn_b - mean*rstd*gn_w)
    a_sb = tp.tile([128, 2], F32, tag="asb")
    nc.vector.tensor_scalar_mul(out=a_sb, in0=gnw_sb, scalar1=bc[:, 1:2])
    b_sb = tp.tile([128, 2], F32, tag="bsb")
    nc.vector.scalar_tensor_tensor(
        out=b_sb, in0=a_sb, scalar=bc[:, 0:1], in1=gnb_sb, op0=OP.mult, op1=OP.add
    )
    for k in range(2):
        nc.vector.tensor_scalar(
            out=xn_sb[:, k, :],
            in0=x_sb[:, k, :],
            scalar1=a_sb[:, k : k + 1],
            scalar2=b_sb[:, k : k + 1],
            op0=OP.mult,
            op1=OP.add,
        )


def _build():
    nc = bacc.Bacc("TRN2", target_bir_lowering=False, debug=False)
    x_d = nc.dram_tensor("x", [SPC, C, HW], F32, kind="ExternalInput").ap()
    qkvwT_d = nc.dram_tensor("qkv_wT", [C, 3 * C], BF16, kind="ExternalInput").ap()
    qbqk_d = nc.dram_tensor("qkv_b_qk", [4, 128], F32, kind="ExternalInput").ap()
    qbv_d = nc.dram_tensor("qkv_b_v", [1, C], F32, kind="ExternalInput").ap()
    pwT128_d = nc.dram_tensor("pwT128", [128, 4 * C], BF16, kind="ExternalInput").ap()
    pb_d = nc.dram_tensor("proj_b", [2, 128], F32, kind="ExternalInput").ap()
    gnw_d = nc.dram_tensor("gn_w", [2, 128], F32, kind="ExternalInput").ap()
    gnb_d = nc.dram_tensor("gn_b", [2, 128], F32, kind="ExternalInput").ap()
    ind66_d = nc.dram_tensor("ind66", [128, 128], BF16, kind="ExternalInput").ap()
    out_d = nc.dram_tensor("out", [SPC, C, HW], F32, kind="ExternalOutput").ap()

    with tile.TileContext(nc) as tc:
        _emit(
            nc, tc, x_d, qkvwT_d, qbqk_d, qbv_d, pwT128_d, pb_d, gnw_d, gnb_d,
            ind66_d, out_d,
        )
    nc.compile()
    return nc


def _emit(
    nc, tc, x_d, qkvwT_d, qbqk_d, qbv_d, pwT128_d, pb_d, gnw_d, gnb_d, ind66_d, out_d
):
    from contextlib import ExitStack

    with ExitStack() as ctx:
        singles = ctx.enter_context(tc.tile_pool(name="singles", bufs=1))
        samp = ctx.enter_context(tc.tile_pool(name="samp", bufs=2))
        o32p = ctx.enter_context(tc.tile_pool(name="o32p", bufs=8))
        tp = ctx.enter_context(tc.tile_pool(name="small", bufs=3))
        e_pool = ctx.enter_context(tc.tile_pool(name="epool", bufs=6))
        ps_st = ctx.enter_context(tc.tile_pool(name="ps_st", bufs=3, space="PSUM"))
        ps_o = ctx.enter_context(tc.tile_pool(name="ps_o", bufs=1, space="PSUM"))
        dr = ctx.enter_context(tc.tile_pool(name="dr", bufs=3, space="DRAM"))
        ps_misc = ps_st  # transient matmul psums share the S^T slots (tag "st")
        pools = {"t": tp, "ps_misc": ps_misc}

        x_tiles = []
        for s in range(SPC):
            x_sb = samp.tile([128, 2, HW], F32, name="x_sb", tag="x")
            nc.sync.dma_start(
                out=x_sb, in_=x_d[s].rearrange("(k p) n -> p k n", p=128)
            )
            x_tiles.append(x_sb)

        # ---- kernel-lifetime constants ----
        qkvwT = singles.tile([128, 2, 3 * C], BF16)
        nc.sync.dma_start(
            out=qkvwT, in_=qkvwT_d.rearrange("(k p) o -> p k o", p=128)
        )
        pwT128 = singles.tile([128, 4, C], BF16)
        nc.sync.dma_start(out=pwT128, in_=pwT128_d.rearrange("p (b o) -> p b o", b=4))
        qb_sb = singles.tile([128, 4], F32)
        nc.sync.dma_start(out=qb_sb, in_=qbqk_d.rearrange("t p -> p t"))
        pb_sb = singles.tile([128, 2], F32)
        nc.sync.dma_start(out=pb_sb, in_=pb_d.rearrange("t p -> p t"))
        gnw_sb = singles.tile([128, 2], F32)
        nc.sync.dma_start(out=gnw_sb, in_=gnw_d.rearrange("t p -> p t"))
        gnb_sb = singles.tile([128, 2], F32)
        nc.sync.dma_start(out=gnb_sb, in_=gnb_d.rearrange("t p -> p t"))
        qbv_sb = singles.tile([1, C], F32)
        nc.sync.dma_start(out=qbv_sb, in_=qbv_d)
        ind66_sb = singles.tile([128, 128], BF16)
        nc.sync.dma_start(out=ind66_sb, in_=ind66_d)
        zeros_col = singles.tile([128, 1], F32)
        nc.vector.memset(zeros_col, 0.0)
        # dummy bf16 matmul burst: pre-warms the HAM clock gate during x DMA
        db = singles.tile([128, 512], BF16)
        nc.vector.memset(db, 0.5)
        for _i in range(16):
            pd = ps_misc.tile([64, 512], F32, name="pd", tag="st")
            nc.tensor.matmul(
                out=pd,
                lhsT=db[:, 0:64],
                rhs=db,
                start=True,
                stop=True,
                skip_group_check=True,
            )
        kpad = []
        if not ST_ROWTILED:
            for i in range(4):
                kp = singles.tile([128, HW], BF16, name=f"kpad{i}")
                nc.vector.tensor_copy(out=kp, in_=zeros_col.to_broadcast([128, HW]))
                kpad.append(kp)
        ones_col = singles.tile([128, 1], F32)
        nc.vector.memset(ones_col, 1.0)
        ones_row = singles.tile([1, 128], F32)
        nc.vector.memset(ones_row, 1.0)
        # o32 pair tiles and rsi slots are allocated up-front but their
        # zero-fills are emitted after qkv(0) (emit_fills) so the early DVE
        # queue belongs to GroupNorm
        o32_tiles = [
            o32p.tile([128, HW], BF16, name=f"o32_{i}", tag="o32")
            for i in range(SPC * 4)
        ]
        rsi_slots = [
            singles.tile([128, HW], BF16, name=f"rsi{i}") for i in range(2)
        ]

        def emit_fills():
            # pre-zero so proj / the ind2 broadcast never see NaN bit patterns
            for ot in o32_tiles:
                nc.vector.tensor_copy(out=ot, in_=zeros_col.to_broadcast([128, HW]))
            for rs in rsi_slots:
                nc.vector.tensor_copy(out=rs, in_=zeros_col.to_broadcast([128, HW]))
        # dummy exp: pulls the ~2.7us ACT table load off the critical path
        dummy_e = tp.tile([1, 8], F32, name="dummy_e", tag="de")
        nc.scalar.activation(out=dummy_e, in_=ones_row[:, 0:8], func=AF.Exp, scale=0.01)
        # broadcast of the v-part qkv bias along partitions: [128, 256]
        vb_ps = ps_misc.tile([128, C], F32, tag="st")
        nc.tensor.matmul(out=vb_ps, lhsT=ones_row, rhs=qbv_sb, start=True, stop=True)
        vb_bc = singles.tile([128, C], F32)
        nc.vector.tensor_copy(out=vb_bc, in_=vb_ps)

        xn_tiles = {}

        def emit_gn(s):
            xn_sb = samp.tile([128, 2, HW], BF16, name="xn_sb", tag="xn")
            _emit_gn(
                nc, pools, x_tiles[s], xn_sb, gnw_sb, gnb_sb, ones_col, ones_row
            )
            xn_tiles[s] = xn_sb

        def emit_qkv(s):
            xn_sb = xn_tiles[s]
            # re-warm the HAM clock gate after the PE-idle GroupNorm stretch
            for _i in range(8):
                pd = ps_misc.tile([64, 512], F32, name="pd2", tag="st")
                nc.tensor.matmul(
                    out=pd,
                    lhsT=xn_sb[:, 0, 0:64],
                    rhs=xn_sb[:, 0, 0:512],
                    start=True,
                    stop=True,
                    skip_group_check=True,
                )
            qk_sb = samp.tile([128, 4, HW], BF16, name="qk_sb", tag="qk")
            for mt in (0, 2, 1, 3):
                for hf in range(2):
                    ps = ps_misc.tile([128, 512], F32, name="ps_q", tag="st")
                    for kc in range(2):
                        nc.tensor.matmul(
                            out=ps,
                            lhsT=qkvwT[:, kc, 128 * mt : 128 * mt + 128],
                            rhs=xn_sb[:, kc, 512 * hf : 512 * hf + 512],
                            start=(kc == 0),
                            stop=(kc == 1),
                            skip_group_check=True,
                        )
                    nc.vector.tensor_scalar_add(
                        out=qk_sb[:, mt, 512 * hf : 512 * hf + 512],
                        in0=ps,
                        scalar1=qb_sb[:, mt : mt + 1],
                    )
            vn_sb = samp.tile([128, 8, NH, HD + 1], BF16, name="vn_sb", tag="vn")
            nc.vector.tensor_copy(
                out=vn_sb[:, :, :, HD : HD + 1],
                in_=ones_col.to_broadcast([128, 8, NH, 1]),
            )
            for j in range(8):
                ps = ps_misc.tile([128, C], F32, name="ps_v", tag="st")
                for kc in range(2):
                    nc.tensor.matmul(
                        out=ps,
                        lhsT=xn_sb[:, kc, 128 * j : 128 * j + 128],
                        rhs=qkvwT[:, kc, 2 * C : 3 * C],
                        start=(kc == 0),
                        stop=(kc == 1),
                        skip_group_check=True,
                    )
                nc.vector.tensor_add(
                    out=vn_sb[:, j, :, 0:HD],
                    in0=ps.rearrange("p (h d) -> p h d", h=NH),
                    in1=vb_bc.rearrange("p (h d) -> p h d", h=NH),
                )
            return qk_sb, vn_sb

        state = {}

        def emit_pair(s, pr):
            qk_sb, vn_sb = state[s]["qkv"]
            heads = (2 * pr, 2 * pr + 1)
            mq = pr // 2
            mk = 2 + mq
            if not ST_ROWTILED:
                for h in heads:
                    qbase = 32 * (h % 4)
                    nc.sync.dma_start(
                        out=kpad[h % 4][qbase : qbase + 32, :],
                        in_=qk_sb[qbase : qbase + 32, mk, :],
                    )
            o_ps = ps_o.tile([128, HW], F32, name="o_ps", tag="o")
            state[s]["ops"][pr] = o_ps
            for j in range(8):
                st_tiles = {}
                for h in heads:
                    band = h % 4
                    st = ps_st.tile([128, HW], F32, name="st", tag="st")
                    st_tiles[h] = st
                    for hf in range(2):
                        if ST_ROWTILED:
                            nc.tensor.matmul(
                                out=st[:, 512 * hf : 512 * hf + 512],
                                lhsT=qk_sb[
                                    32 * band : 32 * band + 32,
                                    mk,
                                    128 * j : 128 * j + 128,
                                ],
                                rhs=qk_sb[
                                    32 * band : 32 * band + 32,
                                    mq,
                                    512 * hf : 512 * hf + 512,
                                ],
                                start=True,
                                stop=True,
                                tile_position=(32 * band, 0),
                                skip_group_check=True,
                            )
                        else:
                            nc.tensor.matmul(
                                out=st[:, 512 * hf : 512 * hf + 512],
                                lhsT=kpad[band][:, 128 * j : 128 * j + 128],
                                rhs=qk_sb[:, mq, 512 * hf : 512 * hf + 512],
                                start=True,
                                stop=True,
                                skip_group_check=True,
                            )
                for t, h in enumerate(heads):
                    e = e_pool.tile([128, HW], BF16, name="e", tag="e")
                    if (j, t) in DVE_SET:
                        nc.vector.tensor_scalar(
                            out=e.bitcast(I16),
                            in0=st_tiles[h],
                            scalar1=A_DVE,
                            scalar2=B_DVE,
                            op0=OP.mult,
                            op1=OP.add,
                        )
                    else:
                        nc.scalar.activation(
                            out=e, in_=st_tiles[h], func=AF.Exp, scale=SCALE
                        )
                    cg = 64 * t  # column group: head A rows 0-32, head B 64-96
                    for hf in range(2):
                        nc.tensor.matmul(
                            out=o_ps[cg : cg + 33, 512 * hf : 512 * hf + 512],
                            lhsT=vn_sb[:, j, h, :],
                            rhs=e[:, 512 * hf : 512 * hf + 512],
                            start=(j == 0),
                            stop=(j == 7),
                            tile_position=(0, cg),
                            skip_group_check=True,
                        )

        def emit_pair_tail(s, pr):
            """Evacuate o_ps (data + den rows) and start the reciprocal chain."""
            o_ps = state[s]["ops"][pr]
            ot = o32_tiles[4 * s + pr]
            state[s]["o32"][pr] = ot
            # rows 0-31 head A data, 32 den A, 64-95 head B data, 96 den B
            nc.vector.tensor_copy(out=ot[0:33, :], in_=o_ps[0:33, :])
            nc.vector.tensor_copy(out=ot[64:97, :], in_=o_ps[64:97, :])
            dden = dr.tile([2, HW], BF16, name="dden", tag="dden")
            nc.sync.dma_start(out=dden[0:1, :], in_=ot[32:33, :])
            nc.sync.dma_start(out=dden[1:2, :], in_=ot[96:97, :])
            dn32 = tp.tile([32, 64], BF16, name="dn32", tag="dn32")
            nc.sync.dma_start(
                out=dn32, in_=dden.rearrange("h (p n) -> (h p) n", p=16)
            )
            rsi32 = tp.tile([32, 64], BF16, name="rsi32", tag="rsi32")
            with nc.allow_low_precision(reason="softmax denom recip in bf16"):
                nc.vector.reciprocal(out=rsi32, in_=dn32)
            drsi = dr.tile([2, HW], BF16, name="drsi", tag="drsi")
            nc.sync.dma_start(
                out=drsi.rearrange("h (p n) -> (h p) n", p=16), in_=rsi32
            )
            rslot = rsi_slots[(4 * s + pr) % 2]
            nc.sync.dma_start(out=rslot[0:2, :], in_=drsi)
            state[s]["rsi"][pr] = rslot

        def emit_norm(s, pr):
            ot = state[s]["o32"][pr]
            rslot = state[s]["rsi"][pr]
            for hf in range(2):
                bc = ps_misc.tile([128, 512], F32, name="bc", tag="st")
                nc.tensor.matmul(
                    out=bc,
                    lhsT=ind66_sb,
                    rhs=rslot[:, 512 * hf : 512 * hf + 512],
                    start=True,
                    stop=True,
                    skip_group_check=True,
                )
                nc.vector.tensor_mul(
                    out=ot[:, 512 * hf : 512 * hf + 512],
                    in0=ot[:, 512 * hf : 512 * hf + 512],
                    in1=bc,
                )

        def emit_proj(s):
            x_sb = x_tiles[s]
            out_sb = samp.tile([128, 2, HW], F32, name="out_sb", tag="outsb")
            for mt in range(2):
                for hf in range(2):
                    ps = ps_misc.tile([128, 512], F32, name="ps_p", tag="st")
                    for b in range(4):
                        nc.tensor.matmul(
                            out=ps,
                            lhsT=pwT128[:, b, 128 * mt : 128 * mt + 128],
                            rhs=state[s]["o32"][b][:, 512 * hf : 512 * hf + 512],
                            start=(b == 0),
                            stop=(b == 3),
                            skip_group_check=True,
                        )
                    nc.vector.scalar_tensor_tensor(
                        out=out_sb[:, mt, 512 * hf : 512 * hf + 512],
                        in0=ps,
                        scalar=pb_sb[:, mt : mt + 1],
                        in1=x_sb[:, mt, 512 * hf : 512 * hf + 512],
                        op0=OP.add,
                        op1=OP.add,
                    )
                    nc.sync.dma_start(
                        out=out_d[s].rearrange("(k p) n -> p k n", p=128)[
                            :, mt, 512 * hf : 512 * hf + 512
                        ],
                        in_=out_sb[:, mt, 512 * hf : 512 * hf + 512],
                    )

        for s in range(SPC):
            state[s] = {"ops": {}, "o32": {}, "rsi": {}}

        # interleaved schedule: sample 1's GN/qkv fills engine gaps during
        # sample 0's attention; each pair's normalize chain trails one pair
        emit_gn(0)
        state[0]["qkv"] = emit_qkv(0)
        emit_fills()
        emit_pair(0, 0)
        emit_pair_tail(0, 0)
        emit_gn(1)
        emit_pair(0, 1)
        emit_pair_tail(0, 1)
        emit_norm(0, 0)
        state[1]["qkv"] = emit_qkv(1)
        emit_pair(0, 2)
        emit_pair_tail(0, 2)
        emit_norm(0, 1)
        emit_pair(0, 3)
        emit_pair_tail(0, 3)
        emit_norm(0, 2)
        emit_norm(0, 3)
        emit_proj(0)
        emit_pair(1, 0)
        emit_pair_tail(1, 0)
        emit_pair(1, 1)
        emit_pair_tail(1, 1)
        emit_norm(1, 0)
        emit_pair(1, 2)
        emit_pair_tail(1, 2)
        emit_norm(1, 1)
        emit_pair(1, 3)
        emit_pair_tail(1, 3)
        emit_norm(1, 2)
        emit_norm(1, 3)
        emit_proj(1)


def _get_nc():
    if "nc" not in _CACHE:
        _CACHE["nc"] = _build()
    return _CACHE["nc"]


def _prep_shared(gn_w, gn_b, qkv_w, qkv_b, proj_w, proj_b):
    qkv_wT = np.ascontiguousarray(
        np.asarray(qkv_w, np.float32).T.astype(ml_dtypes.bfloat16)
    )
    proj_w = np.asarray(proj_w, np.float32)
    # pwT128[r, b, o]: rows 0-31 -> proj channels 64b+r, rows 64-95 ->
    # channels 64b+32+(r-64); den/junk rows (32-63, 96-127) are zero
    pwT128 = np.zeros((128, 4, C), np.float32)
    for b_ in range(4):
        pwT128[0:32, b_, :] = proj_w[:, 64 * b_ : 64 * b_ + 32].T
        pwT128[64:96, b_, :] = proj_w[:, 64 * b_ + 32 : 64 * b_ + 64].T
    qkv_b = np.asarray(qkv_b, np.float32)
    return {
        "qkv_wT": qkv_wT,
        "qkv_b_qk": np.ascontiguousarray(qkv_b[: 2 * C].reshape(4, 128)),
        "qkv_b_v": np.ascontiguousarray(qkv_b[2 * C :].reshape(1, C)),
        "pwT128": np.ascontiguousarray(
            pwT128.reshape(128, 4 * C).astype(ml_dtypes.bfloat16)
        ),
        "proj_b": np.ascontiguousarray(np.asarray(proj_b, np.float32).reshape(2, 128)),
        "gn_w": np.ascontiguousarray(np.asarray(gn_w, np.float32).reshape(2, 128)),
        "gn_b": np.ascontiguousarray(np.asarray(gn_b, np.float32).reshape(2, 128)),
        "ind66": _IND2.astype(ml_dtypes.bfloat16),
    }


def kernel(x, gn_w, gn_b, qkv_w, qkv_b, proj_w, proj_b, **_ignored):
    nc = _get_nc()
    x = np.asarray(x, dtype=np.float32).reshape(B, C, HW)
    shared = _prep_shared(gn_w, gn_b, qkv_w, qkv_b, proj_w, proj_b)
    in_maps = [
        {"x": np.ascontiguousarray(x[i * SPC : (i + 1) * SPC]), **shared}
        for i in range(NCORES)
    ]
    br = run_bass_kernel_spmd(nc, in_maps, core_ids=list(range(NCORES)))
    out = np.concatenate([r["out"] for r in br.results], axis=0)
    return out.reshape(B, C, 32, 32)
